# revision 16
# baseline (speedup 1.0000x reference)
"""DGI (2-layer GCN encoder + bilinear discriminator) on 8 TRN2 NeuronCores.

v2 design. Nodes are assigned to (core, block, slot) positions with a
degree-balanced round-robin deal (high in-degree nodes dealt first across the
8*NB bins), which keeps per-(block, bucket) edge counts near-uniform so the
128-edge chunk count K is ~4 everywhere. Self-loops are real edges, so there
is no separate self-row path. Per layer, each core computes its shard of
h @ W (pos|neg fused on the feature axis), the full feature table is
AllGathered in 4 bucket slices (<=32K rows each for int16 gather indices),
then the segment-sum runs as dma_gather row fetches x one-hot selection
matrices accumulated in PSUM by the PE. Gathers are grouped per (block-pair,
bucket) to amortize the ~1us fixed SWDGE descriptor-generation cost; padded
slots index row 0 (their one-hot rows are zero), so no truncation registers
or warm-up passes are needed. The layer-1 table is fp8 e4m3 and the one-hot
matrices are e4m3 too, so layer-1 aggregation matmuls run in DoubleRow perf
mode (2 chunks / instruction at 0.5 cyc/row); layer-2 stays bf16 (e4m3 there
pushes quantization error past the error budget). The GCN edge norm is
factorized into the tables: x rows are pre-scaled by dinv on the host, the
layer-2 table rows are scaled by dinv^2 (postponing layer-1's dinv_dst scale
through the relu and W2), and each layer's output is post-scaled by dinv on
the Activation engine, which also runs all other per-block post ops (relu,
casts, score bias) to keep DVE and the DMA-heavy engines free. The
transposed H needed by the bilinear scores is cached in SBUF during layer-2
so the score phase is just two matvecs per block.
"""
import sys
sys.path.insert(0, "/opt/trn_rl_repo")

import numpy as np
import concourse.bass as bass
import concourse.bacc as bacc
import concourse.tile as tile
from concourse import bass_utils, mybir
from concourse.masks import make_identity

F32 = mybir.dt.float32
BF16 = mybir.dt.bfloat16
I16 = mybir.dt.int16
F8E4 = mybir.dt.float8e4
F8E3 = mybir.dt.float8e3

BUCK = 4             # table buckets (each <= 32K rows for int16 indices)
MG = 4               # blocks per meta (M/idx) load group
NCAP = 20            # max 128-row chunks per dma_gather (desc-ring headroom)
SP_MODE = 0          # single_packet: 0=auto (<=1024 idxs), 1=always, 2=never
MPP_BUFS = 3
GAT_BUFS = 6
SP_BUFS = 3
PSA_BUFS = 3
XSPAN = 4
DR = False           # DoubleRow fp8 matmuls (requires e4m3 tables)
L2_F8 = True         # layer-2 table in fp8 (upscaled e3m4)
UP2 = 16.0           # layer-2 table upscale (folded out of dinv post-scale)
LOCAL_SIM = False    # replace collectives with local copies (TimelineSim)
ABLATE = ""          # "", "nogather" (skip gathers+matmuls), "nomm" (skip matmuls)


def _cdiv(a, b):
    return -(-a // b)


# ----------------------------------------------------------------------------
# host-side preprocessing
# ----------------------------------------------------------------------------

def _prep(x, edge_index, perm, C):
    N, IN = x.shape
    E = edge_index.shape[1]
    assert N % C == 0
    SH = N // C
    # ~14% slot slack so balanced (block, bucket) edge counts stay under the
    # 4-chunk (512-edge) ceiling; NB multiple of 4 aligns buckets to blocks.
    NB = _cdiv(_cdiv(SH, 128) * 8, 7)
    NB = _cdiv(NB, 4) * 4
    NP = NB * 128
    QB = NP // BUCK
    BS = C * QB
    assert BS <= 32704

    src = np.asarray(edge_index[0], dtype=np.int64)
    dst = np.asarray(edge_index[1], dtype=np.int64)
    perm = np.asarray(perm, dtype=np.int64)

    deg = (1.0 + np.bincount(dst, minlength=N)).astype(np.float32)
    dinv = (1.0 / np.sqrt(deg)).astype(np.float32)

    # node -> (core, block, slot): deal nodes in descending-degree order
    # round-robin across all C*NB bins (cores fastest) to balance both core
    # and block load.
    order = np.argsort(-deg, kind="stable")
    nbins = C * NB
    i = np.arange(N, dtype=np.int64)
    core = np.empty(N, np.int64)
    blk = np.empty(N, np.int64)
    slot = np.empty(N, np.int64)
    core[order] = i % C
    blk[order] = (i // C) % NB
    slot[order] = i // nbins
    assert slot.max() < 128
    sl = blk * 128 + slot                   # local row within the core

    # self-loops as edges
    srcA = np.concatenate([src, np.arange(N, dtype=np.int64)])
    dstA = np.concatenate([dst, np.arange(N, dtype=np.int64)])
    EA = srcA.shape[0]

    dc = core[dstA]
    db = blk[dstA]
    dloc = slot[dstA]
    ssl = sl[srcA]
    su = ssl // QB
    srow = (core[srcA] * QB + (ssl - su * QB)).astype(np.int64)

    cnt = np.bincount((dc * NB + db) * BUCK + su,
                      minlength=C * NB * BUCK).reshape(C, NB, BUCK)
    K = _cdiv(cnt, 128).max(axis=0)         # [NB, BUCK]

    # chunk layout ordered by (meta group mg=b//MG, bucket u, block b, chunk
    # k) so a group's bucket-u chunks are contiguous for one grouped gather.
    assert NB % MG == 0
    chunk_off = np.zeros((NB, BUCK), np.int64)
    tot = 0
    for mg in range(NB // MG):
        for u in range(BUCK):
            for b in range(mg * MG, (mg + 1) * MG):
                chunk_off[b, u] = tot
                tot += int(K[b, u])
    TOT = tot

    fgrp = (dc * NB + db) * BUCK + su
    order_e = np.argsort(fgrp, kind="stable")
    fgrp_s = fgrp[order_e]
    gstart = np.concatenate([[0], np.cumsum(np.bincount(fgrp, minlength=C * NB * BUCK))])
    rank = np.arange(EA, dtype=np.int64) - gstart[fgrp_s]
    bu_s = fgrp_s % (NB * BUCK)
    slot_e = chunk_off.reshape(-1)[bu_s] * 128 + rank
    dc_s = fgrp_s // (NB * BUCK)

    idxbuf = np.zeros((C, TOT * 128), np.int16)   # pads index row 0
    idxbuf[dc_s, slot_e] = srow[order_e].astype(np.int16)
    mdt = mybir.dt.np(F8E3)
    mbuf = np.zeros((C, TOT * 128, 128), mdt)
    mbuf[dc_s, slot_e, dloc[order_e]] = mdt(1.0)  # pads stay zero rows

    idx_dev = np.tile(
        idxbuf.reshape(C, TOT, 8, 16).transpose(0, 3, 1, 2).reshape(C, 16, TOT * 8),
        (1, 8, 1),
    )  # [C, 128, TOT*8]
    m_dev = np.ascontiguousarray(
        mbuf.reshape(C, TOT, 128, 128).transpose(0, 2, 1, 3)
        .reshape(C, 128, TOT * 128))

    # per-position dinv / dinv^2 / valid, [C, 128, NB] (partition = slot)
    dvfull = np.zeros(C * NP, np.float32)
    dvfull[core * NP + sl] = dinv
    dv_raw = dvfull.reshape(C, NB, 128).transpose(0, 2, 1).copy()
    dv2_dev = (dv_raw * dv_raw) * UP2     # layer-2 table write scale
    dv_dev = dv_raw / UP2                 # layer-2 output scale
    vdfull = np.zeros(C * NP, np.float32)
    vdfull[core * NP + sl] = 1.0
    vd_dev = vdfull.reshape(C, NB, 128).transpose(0, 2, 1).astype(mybir.dt.np(BF16))

    # x tables, dinv pre-folded, bf16, feature-major [C, IN, NP]
    xdt = mybir.dt.np(BF16)
    xs = (x * dinv[:, None])
    xn = (x[perm] * dinv[:, None])
    xT_pos = np.zeros((C, IN, NP), xdt)
    xT_neg = np.zeros((C, IN, NP), xdt)
    xT_pos[core, :, sl] = xs.astype(xdt)
    xT_neg[core, :, sl] = xn.astype(xdt)

    meta = dict(N=N, E=E, IN=IN, SH=SH, NB=NB, NP=NP, QB=QB, BS=BS, TOT=TOT,
                K=K, chunk_off=chunk_off, core=core, blk=blk, slot=slot)
    arrays = dict(idx_dev=idx_dev, m_dev=m_dev, dv_dev=dv_dev,
                  dv2_dev=dv2_dev, vd_dev=vd_dev, xT_pos=xT_pos, xT_neg=xT_neg)
    return meta, arrays


# ----------------------------------------------------------------------------
# device program
# ----------------------------------------------------------------------------

def _build(meta, HID, OUT, bias1_nz, bias2_nz, bb_val, C):
    N, IN = meta["N"], meta["IN"]
    NB, NP, QB, BS = meta["NB"], meta["NP"], meta["QB"], meta["BS"]
    TOT = meta["TOT"]
    K, chunk_off = meta["K"], meta["chunk_off"]
    KI, KH = IN // 128, HID // 128
    assert OUT == 128, "discriminator path assumes OUT == 128"
    F1, F2 = 2 * HID, 2 * OUT
    TDT1 = F8E3          # layer-1 table dtype
    GDT2 = F8E3 if L2_F8 else BF16   # layer-2 table dtype
    XDT = BF16
    HDT = BF16

    # gather runs per (meta group, bucket): contiguous chunk spans of <= NCAP
    NG = NB // MG
    mg_runs = [[None] * BUCK for _ in range(NG)]
    nmax = 1
    for mgi in range(NG):
        bs = range(mgi * MG, (mgi + 1) * MG)
        for u in range(BUCK):
            n = sum(int(K[b, u]) for b in bs)
            runs = []
            s = 0
            while s < n:
                rn = min(NCAP, n - s)
                runs.append((s, rn))
                s += rn
            mg_runs[mgi][u] = runs
            nmax = max(nmax, n)
    # meta (M/idx) group column extents
    mg_cols = []
    for b0 in range(0, NB, MG):
        be = min(NB, b0 + MG)
        c0 = int(chunk_off[b0, 0])
        c1 = TOT if be == NB else int(chunk_off[be, 0])
        mg_cols.append(c1 - c0)
    MGC = max(mg_cols)

    nc = bacc.Bacc("TRN2", target_bir_lowering=False, debug=False, num_devices=C,
                   num_swdge_queues=4, dynamic_dma_scratch_size=49152)

    # inputs
    xtp = nc.dram_tensor("xtp", [IN, NP], XDT, kind="ExternalInput")
    xtn = nc.dram_tensor("xtn", [IN, NP], XDT, kind="ExternalInput")
    w1 = nc.dram_tensor("w1", [IN, HID], XDT, kind="ExternalInput")
    w2 = nc.dram_tensor("w2", [HID, OUT], XDT, kind="ExternalInput")
    wbt = nc.dram_tensor("wbt", [OUT, OUT], F32, kind="ExternalInput")
    idx_in = nc.dram_tensor("idx16", [128, TOT * 8], I16, kind="ExternalInput")
    m_in = nc.dram_tensor("monehot", [128, TOT * 128], F8E3, kind="ExternalInput")
    dv_in = nc.dram_tensor("dinv", [128, NB], F32, kind="ExternalInput")
    dv2_in = nc.dram_tensor("dinv2", [128, NB], F32, kind="ExternalInput")
    vd_in = nc.dram_tensor("valid", [128, NB], HDT, kind="ExternalInput")
    b1_in = nc.dram_tensor("b1bc", [128, F1], F32, kind="ExternalInput") if bias1_nz else None
    b2_in = nc.dram_tensor("b2bc", [128, F2], F32, kind="ExternalInput") if bias2_nz else None
    out = nc.dram_tensor("scores", [2, 128, NB], F32, kind="ExternalOutput")

    # internal DRAM
    hw1t_sh = nc.dram_tensor("hw1t_sh", [NP, F1], TDT1, kind="Internal")
    hw2_sh = nc.dram_tensor("hw2_sh", [NP, F2], GDT2, kind="Internal")
    hw1_full = [nc.dram_tensor(f"hw1_full{j}", [BS, F1], TDT1, kind="Internal",
                               addr_space="Shared") for j in range(BUCK)]
    hw2_full = [nc.dram_tensor(f"hw2_full{j}", [BS, F2], GDT2, kind="Internal",
                               addr_space="Shared") for j in range(BUCK)]
    h2d = nc.dram_tensor("h2d", [128, NB * F2], GDT2 if False else BF16,
                         kind="Internal")
    cs_in = nc.dram_tensor("cs_in", [128, 1], F32, kind="Internal")
    cs_out = nc.dram_tensor("cs_out", [128, 1], F32, kind="Internal",
                            addr_space="Shared")

    ACT = mybir.ActivationFunctionType

    with tile.TileContext(nc) as tc:
        with tc.tile_pool(name="const", bufs=1) as cp, \
             tc.tile_pool(name="stream", bufs=SP_BUFS) as sp, \
             tc.tile_pool(name="meta", bufs=MPP_BUFS) as mpp, \
             tc.tile_pool(name="gat", bufs=GAT_BUFS) as gp, \
             tc.tile_pool(name="psA", bufs=PSA_BUFS, space="PSUM") as psA, \
             tc.tile_pool(name="psT", bufs=2, space="PSUM") as psT, \
             tc.tile_pool(name="psH", bufs=2, space="PSUM") as psH, \
             tc.tile_pool(name="psC", bufs=1, space="PSUM") as psC:

            def allgather(shard, fulls, F, DTY):
                for j in range(BUCK):
                    if LOCAL_SIM:
                        for i in range(QB // 128):
                            tcp = sp.tile([128, F], DTY, tag="agcopy",
                                          name=f"agc_{shard.name}_{j}_{i}")
                            nc.sync.dma_start(
                                out=tcp[:],
                                in_=shard[j * QB + i * 128:j * QB + (i + 1) * 128, :])
                            nc.sync.dma_start(
                                out=fulls[j][i * 128:(i + 1) * 128, :], in_=tcp[:])
                    else:
                        nc.gpsimd.collective_compute(
                            "AllGather", mybir.AluOpType.bypass,
                            replica_groups=[list(range(C))],
                            ins=[shard[j * QB:(j + 1) * QB, :].opt()],
                            outs=[fulls[j][:, :].opt()])

            # constants
            identb = cp.tile([128, 128], BF16)
            make_identity(nc, identb[:])
            identf = cp.tile([128, 128], F32)
            make_identity(nc, identf[:])
            w1sb = cp.tile([128, KI, HID], XDT)
            for k in range(KI):
                nc.sync.dma_start(out=w1sb[:, k, :], in_=w1[k * 128:(k + 1) * 128, :])
            w2sb = cp.tile([128, KH, OUT], XDT)
            for k in range(KH):
                nc.sync.dma_start(out=w2sb[:, k, :], in_=w2[k * 128:(k + 1) * 128, :])
            wbtsb = cp.tile([128, OUT], F32)
            nc.sync.dma_start(out=wbtsb[:], in_=wbt[:, :])
            dvsb = cp.tile([128, NB], F32)
            nc.sync.dma_start(out=dvsb[:], in_=dv_in[:, :])
            dv2sb = cp.tile([128, NB], F32)
            nc.sync.dma_start(out=dv2sb[:], in_=dv2_in[:, :])
            vdsb = cp.tile([128, NB], HDT)
            nc.sync.dma_start(out=vdsb[:], in_=vd_in[:, :])
            b1sb = b2sb = None
            if bias1_nz:
                b1sb = cp.tile([128, F1], F32)
                nc.sync.dma_start(out=b1sb[:], in_=b1_in[:, :])
            if bias2_nz:
                b2sb = cp.tile([128, F2], F32)
                nc.sync.dma_start(out=b2sb[:], in_=b2_in[:, :])
            sc_pos = cp.tile([128, NB], F32, tag="scp")
            sc_neg = cp.tile([128, NB], F32, tag="scn")
            vbc = cp.tile([128, 128], F32, tag="vbc")

            # ---------------- phase A: hw1 = (dinv*x) @ W1 (pos|neg) ---------
            for sb0 in range(0, NB, XSPAN):
                span = min(XSPAN, NB - sb0)
                xp = sp.tile([128, KI, XSPAN * 128], XDT, tag="xtp")
                xn_t = sp.tile([128, KI, XSPAN * 128], XDT, tag="xtn")
                for k in range(KI):
                    nc.sync.dma_start(
                        out=xp[:, k, :span * 128],
                        in_=xtp[k * 128:(k + 1) * 128, sb0 * 128:(sb0 + span) * 128])
                    nc.sync.dma_start(
                        out=xn_t[:, k, :span * 128],
                        in_=xtn[k * 128:(k + 1) * 128, sb0 * 128:(sb0 + span) * 128])
                for j in range(span):
                    nb_ = sb0 + j
                    pa = psA.tile([128, F1], F32, tag="agg", space="PSUM")
                    for k in range(KI):
                        nc.tensor.matmul(
                            out=pa[:, 0:HID],
                            lhsT=xp[:, k, j * 128:(j + 1) * 128],
                            rhs=w1sb[:, k, :],
                            start=(k == 0), stop=(k == KI - 1))
                    for k in range(KI):
                        nc.tensor.matmul(
                            out=pa[:, HID:F1],
                            lhsT=xn_t[:, k, j * 128:(j + 1) * 128],
                            rhs=w1sb[:, k, :],
                            start=(k == 0), stop=(k == KI - 1))
                    hw1sb = sp.tile([128, F1], TDT1, tag="hw1sb")
                    nc.scalar.activation(out=hw1sb[:], in_=pa[:], func=ACT.Copy)
                    nc.sync.dma_start(out=hw1t_sh[nb_ * 128:(nb_ + 1) * 128, :],
                                      in_=hw1sb[:])

            allgather(hw1t_sh, hw1_full, F1, TDT1)

            # ---------------- aggregation layers ----------------
            def agg_layer(layer):
                F = F1 if layer == 1 else F2
                DTY = TDT1 if layer == 1 else GDT2
                fulls = hw1_full if layer == 1 else hw2_full
                bsb = b1sb if layer == 1 else b2sb
                use_dr = DR
                for mgi in range(NB // MG):
                    b0 = mgi * MG
                    mg0 = int(chunk_off[b0, 0])
                    gcols = mg_cols[mgi]
                    mt = mpp.tile([128, MGC, 128], F8E3, tag="mt",
                                  name=f"mt{layer}_{b0}")
                    ix = mpp.tile([128, MGC * 8], I16, tag="ix",
                                  name=f"ix{layer}_{b0}")
                    nc.scalar.dma_start(
                        out=mt[:, :gcols, :],
                        in_=m_in[:, mg0 * 128:(mg0 + gcols) * 128])
                    nc.scalar.dma_start(
                        out=ix[:, :gcols * 8],
                        in_=idx_in[:, mg0 * 8:(mg0 + gcols) * 8])
                    # grouped gathers for this meta group
                    gts = [None] * BUCK
                    if ABLATE != "nogather":
                        for u in range(BUCK):
                            runs = mg_runs[mgi][u]
                            if not runs:
                                continue
                            co = int(chunk_off[b0, u])
                            gtv = gp.tile([128, nmax, F], DTY,
                                          tag="gat", name=f"gt{layer}_{mgi}_{u}")
                            for (rs, rn) in runs:
                                nc.gpsimd.dma_gather(
                                    out_ap=gtv[:, rs:rs + rn, :],
                                    in_ap=fulls[u][:, :],
                                    idxs_ap=ix[:, (co - mg0 + rs) * 8:
                                               (co - mg0 + rs + rn) * 8],
                                    num_idxs=rn * 128,
                                    num_idxs_reg=rn * 128,
                                    elem_size=F,
                                    single_packet=(
                                        True if SP_MODE == 1 else
                                        False if SP_MODE == 2 else
                                        rn * 128 <= 1024),
                                    queue_num=u)
                            gts[u] = gtv
                    for b in range(b0, b0 + MG):
                        kb = 0 if ABLATE == "nogather" else int(K[b].sum())
                        ps_agg = psA.tile([128, F1], F32, tag="agg", space="PSUM")
                        if kb > 0 and ABLATE != "nomm":
                            # count instructions for start/stop flags
                            insts = []
                            for u in range(BUCK):
                                ku = int(K[b, u])
                                if ku == 0:
                                    continue
                                lo = int(chunk_off[b, u]) - mg0
                                go = int(chunk_off[b, u]) - int(chunk_off[b0, u])
                                j = 0
                                while j < ku:
                                    step = 2 if (use_dr and j + 2 <= ku) else 1
                                    insts.append((u, lo + j, go + j, step))
                                    j += step
                            for t, (u, lo, go, step) in enumerate(insts):
                                if step == 2:
                                    nc.tensor.matmul(
                                        out=ps_agg[:, :F],
                                        lhsT=mt[:, lo:lo + 2, :],
                                        rhs=gts[u][:, go:go + 2, :],
                                        perf_mode=mybir.MatmulPerfMode.DoubleRow,
                                        start=(t == 0), stop=(t == len(insts) - 1))
                                else:
                                    nc.tensor.matmul(
                                        out=ps_agg[:, :F],
                                        lhsT=mt[:, lo, :],
                                        rhs=gts[u][:, go, :],
                                        start=(t == 0), stop=(t == len(insts) - 1))
                        else:
                            nc.vector.memset(ps_agg[:], 0.0)

                        if layer == 1:
                            hout = sp.tile([128, F1], XDT, tag="hout",
                                           name=f"ho1_{b}")
                            if bsb is None:
                                nc.scalar.activation(out=hout[:], in_=ps_agg[:],
                                                     func=ACT.Relu)
                            else:
                                nc.scalar.activation(
                                    out=hout[:], in_=ps_agg[:], func=ACT.Copy,
                                    scale=dvsb[:, b:b + 1])
                                nc.vector.tensor_tensor(
                                    out=hout[:], in0=hout[:], in1=bsb[:],
                                    op=mybir.AluOpType.add)
                                nc.vector.tensor_scalar(
                                    out=hout[:], in0=hout[:], scalar1=0.0,
                                    scalar2=None, op0=mybir.AluOpType.max)
                            ps_tp = psT.tile([128, F1], XDT, tag="tp",
                                             space="PSUM", name=f"tp1_{b}")
                            for k in range(2 * KH):
                                nc.tensor.transpose(
                                    out=ps_tp[:, k * 128:(k + 1) * 128],
                                    in_=hout[:, k * 128:(k + 1) * 128],
                                    identity=identb[:])
                            ts = sp.tile([128, F1], XDT, tag="ts", name=f"ts1_{b}")
                            nc.scalar.copy(out=ts[:], in_=ps_tp[:])
                            ps_h2 = psH.tile([128, F2], F32, tag="h2",
                                             space="PSUM", name=f"h2_{b}")
                            for k in range(KH):
                                nc.tensor.matmul(
                                    out=ps_h2[:, 0:OUT],
                                    lhsT=ts[:, k * 128:(k + 1) * 128],
                                    rhs=w2sb[:, k, :],
                                    start=(k == 0), stop=(k == KH - 1))
                            for k in range(KH):
                                nc.tensor.matmul(
                                    out=ps_h2[:, OUT:F2],
                                    lhsT=ts[:, (KH + k) * 128:(KH + k + 1) * 128],
                                    rhs=w2sb[:, k, :],
                                    start=(k == 0), stop=(k == KH - 1))
                            # layer-2 table rows: dinv^2 * (relu(s1) @ W2)
                            # (postponed layer-1 dinv_dst and layer-2 src factor)
                            hw2sb = sp.tile([128, F2], GDT2, tag="hw2sb",
                                            name=f"hw2sb_{b}")
                            nc.scalar.activation(
                                out=hw2sb[:], in_=ps_h2[:], func=ACT.Copy,
                                scale=dv2sb[:, b:b + 1] if bsb is None
                                else dvsb[:, b:b + 1])
                            nc.sync.dma_start(
                                out=hw2_sh[b * 128:(b + 1) * 128, :],
                                in_=hw2sb[:])
                        else:
                            hout = sp.tile([128, F2], HDT, tag="hout2",
                                           name=f"ho2_{b}")
                            nc.scalar.activation(
                                out=hout[:], in_=ps_agg[:, :F2], func=ACT.Copy,
                                scale=dvsb[:, b:b + 1])
                            if bsb is not None:
                                nc.vector.tensor_tensor(
                                    out=hout[:], in0=hout[:], in1=bsb[:],
                                    op=mybir.AluOpType.add)
                            if b == 0:
                                ps_cs = psC.tile([128, 1], F32, tag="cs",
                                                 space="PSUM")
                                agg_layer.cs = ps_cs
                            else:
                                ps_cs = agg_layer.cs
                            nc.tensor.matmul(
                                out=ps_cs[:], lhsT=hout[:, 0:OUT],
                                rhs=vdsb[:, b:b + 1],
                                start=(b == 0), stop=(b == NB - 1),
                                skip_group_check=True)
                            nc.sync.dma_start(
                                out=h2d[:, b * F2:(b + 1) * F2], in_=hout[:])

            agg_layer(1)
            allgather(hw2_sh, hw2_full, F2, GDT2)
            agg_layer(2)

            # ---------------- summary s and v = Wb @ s ----------------
            cssb = sp.tile([128, 1], F32, tag="cssb")
            nc.scalar.copy(out=cssb[:], in_=agg_layer.cs[:])
            nc.sync.dma_start(out=cs_in[:, :], in_=cssb[:])
            if LOCAL_SIM:
                nc.sync.dma_start(out=cs_out[:, :], in_=cssb[:])
            else:
                nc.gpsimd.collective_compute(
                    "AllReduce", mybir.AluOpType.add,
                    replica_groups=[list(range(C))],
                    ins=[cs_in[:, :].opt()], outs=[cs_out[:, :].opt()])
            csr = sp.tile([128, 1], F32, tag="csr")
            nc.sync.dma_start(out=csr[:], in_=cs_out[:, :])
            ssb = sp.tile([128, 1], F32, tag="ssb")
            nc.scalar.activation(out=ssb[:], in_=csr[:], func=ACT.Sigmoid,
                                 scale=1.0 / N)
            ps_v = psC.tile([128, 1], F32, tag="cs", space="PSUM")
            nc.tensor.matmul(out=ps_v[:], lhsT=wbtsb[:], rhs=ssb[:],
                             start=True, stop=True)
            vsb = sp.tile([128, 1], F32, tag="vsb")
            nc.scalar.copy(out=vsb[:], in_=ps_v[:])
            # v^T broadcast to all partitions: vbc[p, f] = v[f]
            ps_vt = psH.tile([128, F2], F32, tag="h2", space="PSUM")
            nc.tensor.transpose(out=ps_vt[0:1, 0:128], in_=vsb[:, 0:1],
                                identity=identf[:])
            vrow = sp.tile([128, 128], F32, tag="vrow")
            nc.scalar.copy(out=vrow[0:1, :], in_=ps_vt[0:1, 0:128])
            nc.gpsimd.partition_broadcast(out_ap=vbc[:], in_ap=vrow[0:1, :])

            # ---------------- scores: sc[p, b] = H[p, b, :] . v ----------
            SCB = 8
            for sb0 in range(0, NB, SCB):
                span = min(SCB, NB - sb0)
                hld = sp.tile([128, SCB * F2], HDT, tag="hld")
                nc.sync.dma_start(out=hld[:, :span * F2],
                                  in_=h2d[:, sb0 * F2:(sb0 + span) * F2])
                for j in range(span):
                    b = sb0 + j
                    prod = sp.tile([128, F2], F32, tag="prod")
                    nc.vector.tensor_tensor(
                        out=prod[:, 0:128], in0=hld[:, j * F2:j * F2 + 128],
                        in1=vbc[:], op=mybir.AluOpType.mult)
                    nc.vector.tensor_tensor(
                        out=prod[:, 128:F2],
                        in0=hld[:, j * F2 + 128:(j + 1) * F2],
                        in1=vbc[:], op=mybir.AluOpType.mult)
                    nc.vector.reduce_sum(out=sc_pos[:, b:b + 1],
                                         in_=prod[:, 0:128],
                                         axis=mybir.AxisListType.X)
                    nc.vector.reduce_sum(out=sc_neg[:, b:b + 1],
                                         in_=prod[:, 128:F2],
                                         axis=mybir.AxisListType.X)
            if bb_val != 0.0:
                nc.scalar.activation(out=sc_pos[:], in_=sc_pos[:],
                                     func=ACT.Copy, bias=float(bb_val))
                nc.scalar.activation(out=sc_neg[:], in_=sc_neg[:],
                                     func=ACT.Copy, bias=float(bb_val))
            nc.sync.dma_start(out=out[0, :, :], in_=sc_pos[:])
            nc.sync.dma_start(out=out[1, :, :], in_=sc_neg[:])

    nc.compile()
    return nc


# ----------------------------------------------------------------------------
# entry point
# ----------------------------------------------------------------------------

_CACHE = {}


def _get_program(meta, HID, OUT, bias1_nz, bias2_nz, bb_val, C):
    key = (meta["N"], meta["E"], meta["IN"], HID, OUT, bias1_nz, bias2_nz,
           float(bb_val), C, meta["TOT"], meta["K"].tobytes())
    if key not in _CACHE:
        _CACHE[key] = _build(meta, HID, OUT, bias1_nz, bias2_nz, bb_val, C)
    return _CACHE[key]


def _make_in_maps(meta, arrs, W1, b1, W2, b2, Wb, C, bias1_nz, bias2_nz):
    wdt = mybir.dt.np(BF16)
    W1, W2 = W1.astype(wdt), W2.astype(wdt)
    in_maps = []
    for c in range(C):
        m = {
            "xtp": arrs["xT_pos"][c], "xtn": arrs["xT_neg"][c],
            "w1": W1, "w2": W2, "wbt": np.ascontiguousarray(Wb.T),
            "idx16": arrs["idx_dev"][c], "monehot": arrs["m_dev"][c],
            "dinv": arrs["dv_dev"][c], "dinv2": arrs["dv2_dev"][c],
            "valid": arrs["vd_dev"][c],
        }
        if bias1_nz:
            m["b1bc"] = np.tile(np.concatenate([b1, b1])[None, :], (128, 1))
        if bias2_nz:
            m["b2bc"] = np.tile(np.concatenate([b2, b2])[None, :], (128, 1))
        in_maps.append(m)
    return in_maps


def _unpack(meta, results):
    N = meta["N"]
    core, blk, slot = meta["core"], meta["blk"], meta["slot"]
    pos = np.empty((N, 1), np.float32)
    neg = np.empty((N, 1), np.float32)
    sc0 = np.stack([np.asarray(results[c]["scores"][0]) for c in range(len(results))])
    sc1 = np.stack([np.asarray(results[c]["scores"][1]) for c in range(len(results))])
    pos[:, 0] = sc0[core, slot, blk]
    neg[:, 0] = sc1[core, slot, blk]
    return pos, neg


def kernel(x, edge_index, perm, W1, b1, W2, b2, Wb, bb):
    C = 8
    x = np.asarray(x, np.float32)
    W1 = np.asarray(W1, np.float32)
    W2 = np.asarray(W2, np.float32)
    Wb = np.asarray(Wb, np.float32)
    b1 = np.asarray(b1, np.float32)
    b2 = np.asarray(b2, np.float32)
    bb_val = float(np.asarray(bb).reshape(-1)[0])
    HID = W1.shape[1]
    OUT = W2.shape[1]

    meta, arrs = _prep(x, edge_index, perm, C)
    bias1_nz = bool(np.any(b1))
    bias2_nz = bool(np.any(b2))
    nc = _get_program(meta, HID, OUT, bias1_nz, bias2_nz, bb_val, C)
    in_maps = _make_in_maps(meta, arrs, W1, b1, W2, b2, Wb, C, bias1_nz, bias2_nz)

    res = bass_utils.run_bass_kernel_spmd(nc, in_maps, core_ids=list(range(C)))
    return _unpack(meta, res.results)


# revision 17
# speedup vs baseline: 1.3332x; 1.3332x over previous
"""DGI (2-layer GCN encoder + bilinear discriminator) on 8 TRN2 NeuronCores.

v2 design. Nodes are assigned to (core, block, slot) positions with a
degree-balanced round-robin deal (high in-degree nodes dealt first across the
8*NB bins), which keeps per-(block, bucket) edge counts near-uniform so the
128-edge chunk count K is ~4 everywhere. Self-loops are real edges, so there
is no separate self-row path. Per layer, each core computes its shard of
h @ W (pos|neg fused on the feature axis), the full feature table is
AllGathered in 4 bucket slices (<=32K rows each for int16 gather indices),
then the segment-sum runs as dma_gather row fetches x one-hot selection
matrices accumulated in PSUM by the PE. Gathers are grouped per (block-pair,
bucket) to amortize the ~1us fixed SWDGE descriptor-generation cost; padded
slots index row 0 (their one-hot rows are zero), so no truncation registers
or warm-up passes are needed. The layer-1 table is fp8 e4m3 and the one-hot
matrices are e4m3 too, so layer-1 aggregation matmuls run in DoubleRow perf
mode (2 chunks / instruction at 0.5 cyc/row); layer-2 stays bf16 (e4m3 there
pushes quantization error past the error budget). The GCN edge norm is
factorized into the tables: x rows are pre-scaled by dinv on the host, the
layer-2 table rows are scaled by dinv^2 (postponing layer-1's dinv_dst scale
through the relu and W2), and each layer's output is post-scaled by dinv on
the Activation engine, which also runs all other per-block post ops (relu,
casts, score bias) to keep DVE and the DMA-heavy engines free. The
transposed H needed by the bilinear scores is cached in SBUF during layer-2
so the score phase is just two matvecs per block.
"""
import sys
sys.path.insert(0, "/opt/trn_rl_repo")

import numpy as np
import concourse.bass as bass
import concourse.bacc as bacc
import concourse.tile as tile
from concourse import bass_utils, mybir
from concourse.masks import make_identity

F32 = mybir.dt.float32
BF16 = mybir.dt.bfloat16
I16 = mybir.dt.int16
F8E4 = mybir.dt.float8e4
F8E3 = mybir.dt.float8e3

BUCK = 4             # table buckets (each <= 32K rows for int16 indices)
MG = 4               # blocks per meta (M/idx) load group
NCAP = 20            # max 128-row chunks per dma_gather (desc-ring headroom)
GSUB = 2             # blocks per gather group (divides MG)
SP_MODE = 0          # single_packet: 0=auto (<=1024 idxs), 1=always, 2=never
MPP_BUFS = 3
GAT_BUFS = 6
SP_BUFS = 3
PSA_BUFS = 3
XSPAN = 4
DR = False           # DoubleRow fp8 matmuls (requires e4m3 tables)
L2_F8 = True         # layer-2 table in fp8 (upscaled e3m4)
UP2 = 16.0           # layer-2 table upscale (folded out of dinv post-scale)
LOCAL_SIM = False    # replace collectives with local copies (TimelineSim)
ABLATE = ""          # "", "nogather" (skip gathers+matmuls), "nomm" (skip matmuls)


def _cdiv(a, b):
    return -(-a // b)


# ----------------------------------------------------------------------------
# host-side preprocessing
# ----------------------------------------------------------------------------

def _prep(x, edge_index, perm, C):
    N, IN = x.shape
    E = edge_index.shape[1]
    assert N % C == 0
    SH = N // C
    # ~14% slot slack so balanced (block, bucket) edge counts stay under the
    # 4-chunk (512-edge) ceiling; NB multiple of 4 aligns buckets to blocks.
    NB = _cdiv(_cdiv(SH, 128) * 8, 7)
    NB = _cdiv(NB, 4) * 4
    NP = NB * 128
    QB = NP // BUCK
    BS = C * QB
    assert BS <= 32704

    src = np.asarray(edge_index[0], dtype=np.int64)
    dst = np.asarray(edge_index[1], dtype=np.int64)
    perm = np.asarray(perm, dtype=np.int64)

    deg = (1.0 + np.bincount(dst, minlength=N)).astype(np.float32)
    dinv = (1.0 / np.sqrt(deg)).astype(np.float32)

    # node -> (core, block, slot): deal nodes in descending-degree order
    # round-robin across all C*NB bins (cores fastest) to balance both core
    # and block load.
    order = np.argsort(-deg, kind="stable")
    nbins = C * NB
    i = np.arange(N, dtype=np.int64)
    core = np.empty(N, np.int64)
    blk = np.empty(N, np.int64)
    slot = np.empty(N, np.int64)
    core[order] = i % C
    blk[order] = (i // C) % NB
    slot[order] = i // nbins
    assert slot.max() < 128
    sl = blk * 128 + slot                   # local row within the core

    # self-loops as edges
    srcA = np.concatenate([src, np.arange(N, dtype=np.int64)])
    dstA = np.concatenate([dst, np.arange(N, dtype=np.int64)])
    EA = srcA.shape[0]

    dc = core[dstA]
    db = blk[dstA]
    dloc = slot[dstA]
    ssl = sl[srcA]
    su = ssl // QB
    srow = (core[srcA] * QB + (ssl - su * QB)).astype(np.int64)

    cnt = np.bincount((dc * NB + db) * BUCK + su,
                      minlength=C * NB * BUCK).reshape(C, NB, BUCK)
    K = _cdiv(cnt, 128).max(axis=0)         # [NB, BUCK]

    # chunk layout ordered by (meta group mg=b//MG, bucket u, block b, chunk
    # k) so a group's bucket-u chunks are contiguous for one grouped gather.
    assert NB % MG == 0
    chunk_off = np.zeros((NB, BUCK), np.int64)
    tot = 0
    for mg in range(NB // MG):
        for u in range(BUCK):
            for b in range(mg * MG, (mg + 1) * MG):
                chunk_off[b, u] = tot
                tot += int(K[b, u])
    TOT = tot

    fgrp = (dc * NB + db) * BUCK + su
    order_e = np.argsort(fgrp, kind="stable")
    fgrp_s = fgrp[order_e]
    gstart = np.concatenate([[0], np.cumsum(np.bincount(fgrp, minlength=C * NB * BUCK))])
    rank = np.arange(EA, dtype=np.int64) - gstart[fgrp_s]
    bu_s = fgrp_s % (NB * BUCK)
    slot_e = chunk_off.reshape(-1)[bu_s] * 128 + rank
    dc_s = fgrp_s // (NB * BUCK)

    idxbuf = np.zeros((C, TOT * 128), np.int16)   # pads index row 0
    idxbuf[dc_s, slot_e] = srow[order_e].astype(np.int16)
    mdt = mybir.dt.np(F8E3)
    mbuf = np.zeros((C, TOT * 128, 128), mdt)
    mbuf[dc_s, slot_e, dloc[order_e]] = mdt(1.0)  # pads stay zero rows

    idx_dev = np.tile(
        idxbuf.reshape(C, TOT, 8, 16).transpose(0, 3, 1, 2).reshape(C, 16, TOT * 8),
        (1, 8, 1),
    )  # [C, 128, TOT*8]
    m_dev = np.ascontiguousarray(
        mbuf.reshape(C, TOT, 128, 128).transpose(0, 2, 1, 3)
        .reshape(C, 128, TOT * 128))

    # per-position dinv / dinv^2 / valid, [C, 128, NB] (partition = slot)
    dvfull = np.zeros(C * NP, np.float32)
    dvfull[core * NP + sl] = dinv
    dv_raw = dvfull.reshape(C, NB, 128).transpose(0, 2, 1).copy()
    dv2_dev = (dv_raw * dv_raw) * UP2     # layer-2 table write scale
    dv_dev = dv_raw / UP2                 # layer-2 output scale
    vdfull = np.zeros(C * NP, np.float32)
    vdfull[core * NP + sl] = 1.0
    vd_dev = vdfull.reshape(C, NB, 128).transpose(0, 2, 1).astype(mybir.dt.np(BF16))

    # x tables, dinv pre-folded, bf16, feature-major [C, IN, NP]
    xdt = mybir.dt.np(BF16)
    xs = (x * dinv[:, None])
    xn = (x[perm] * dinv[:, None])
    xT_pos = np.zeros((C, IN, NP), xdt)
    xT_neg = np.zeros((C, IN, NP), xdt)
    xT_pos[core, :, sl] = xs.astype(xdt)
    xT_neg[core, :, sl] = xn.astype(xdt)

    meta = dict(N=N, E=E, IN=IN, SH=SH, NB=NB, NP=NP, QB=QB, BS=BS, TOT=TOT,
                K=K, chunk_off=chunk_off, core=core, blk=blk, slot=slot)
    arrays = dict(idx_dev=idx_dev, m_dev=m_dev, dv_dev=dv_dev,
                  dv2_dev=dv2_dev, vd_dev=vd_dev, xT_pos=xT_pos, xT_neg=xT_neg)
    return meta, arrays


# ----------------------------------------------------------------------------
# device program
# ----------------------------------------------------------------------------

def _build(meta, HID, OUT, bias1_nz, bias2_nz, bb_val, C):
    N, IN = meta["N"], meta["IN"]
    NB, NP, QB, BS = meta["NB"], meta["NP"], meta["QB"], meta["BS"]
    TOT = meta["TOT"]
    K, chunk_off = meta["K"], meta["chunk_off"]
    KI, KH = IN // 128, HID // 128
    assert OUT == 128, "discriminator path assumes OUT == 128"
    F1, F2 = 2 * HID, 2 * OUT
    TDT1 = F8E3          # layer-1 table dtype
    GDT2 = F8E3 if L2_F8 else BF16   # layer-2 table dtype
    XDT = BF16
    HDT = BF16

    # gather runs per (gather group of GSUB blocks, bucket): contiguous
    # chunk spans of <= NCAP, offsets relative to the gather group start
    assert MG % GSUB == 0
    NG = NB // GSUB
    gg_runs = [[None] * BUCK for _ in range(NG)]
    nmax = 1
    for gg in range(NG):
        bs = range(gg * GSUB, (gg + 1) * GSUB)
        for u in range(BUCK):
            n = sum(int(K[b, u]) for b in bs)
            runs = []
            s = 0
            while s < n:
                rn = min(NCAP, n - s)
                runs.append((s, rn))
                s += rn
            gg_runs[gg][u] = runs
            nmax = max(nmax, n)
    # meta (M/idx) group column extents
    mg_cols = []
    for b0 in range(0, NB, MG):
        be = min(NB, b0 + MG)
        c0 = int(chunk_off[b0, 0])
        c1 = TOT if be == NB else int(chunk_off[be, 0])
        mg_cols.append(c1 - c0)
    MGC = max(mg_cols)

    nc = bacc.Bacc("TRN2", target_bir_lowering=False, debug=False, num_devices=C,
                   num_swdge_queues=4, dynamic_dma_scratch_size=49152)

    # inputs
    xtp = nc.dram_tensor("xtp", [IN, NP], XDT, kind="ExternalInput")
    xtn = nc.dram_tensor("xtn", [IN, NP], XDT, kind="ExternalInput")
    w1 = nc.dram_tensor("w1", [IN, HID], XDT, kind="ExternalInput")
    w2 = nc.dram_tensor("w2", [HID, OUT], XDT, kind="ExternalInput")
    wbt = nc.dram_tensor("wbt", [OUT, OUT], F32, kind="ExternalInput")
    idx_in = nc.dram_tensor("idx16", [128, TOT * 8], I16, kind="ExternalInput")
    m_in = nc.dram_tensor("monehot", [128, TOT * 128], F8E3, kind="ExternalInput")
    dv_in = nc.dram_tensor("dinv", [128, NB], F32, kind="ExternalInput")
    dv2_in = nc.dram_tensor("dinv2", [128, NB], F32, kind="ExternalInput")
    vd_in = nc.dram_tensor("valid", [128, NB], HDT, kind="ExternalInput")
    b1_in = nc.dram_tensor("b1bc", [128, F1], F32, kind="ExternalInput") if bias1_nz else None
    b2_in = nc.dram_tensor("b2bc", [128, F2], F32, kind="ExternalInput") if bias2_nz else None
    out = nc.dram_tensor("scores", [2, 128, NB], F32, kind="ExternalOutput")

    # internal DRAM
    hw1t_sh = nc.dram_tensor("hw1t_sh", [NP, F1], TDT1, kind="Internal")
    hw2_sh = nc.dram_tensor("hw2_sh", [NP, F2], GDT2, kind="Internal")
    hw1_full = [nc.dram_tensor(f"hw1_full{j}", [BS, F1], TDT1, kind="Internal",
                               addr_space="Shared") for j in range(BUCK)]
    hw2_full = [nc.dram_tensor(f"hw2_full{j}", [BS, F2], GDT2, kind="Internal",
                               addr_space="Shared") for j in range(BUCK)]
    h2d = nc.dram_tensor("h2d", [128, NB * F2], GDT2 if False else BF16,
                         kind="Internal")
    cs_in = nc.dram_tensor("cs_in", [128, 1], F32, kind="Internal")
    cs_out = nc.dram_tensor("cs_out", [128, 1], F32, kind="Internal",
                            addr_space="Shared")

    ACT = mybir.ActivationFunctionType

    with tile.TileContext(nc) as tc:
        with tc.tile_pool(name="const", bufs=1) as cp, \
             tc.tile_pool(name="stream", bufs=SP_BUFS) as sp, \
             tc.tile_pool(name="meta", bufs=MPP_BUFS) as mpp, \
             tc.tile_pool(name="gat", bufs=GAT_BUFS) as gp, \
             tc.tile_pool(name="psA", bufs=PSA_BUFS, space="PSUM") as psA, \
             tc.tile_pool(name="psT", bufs=2, space="PSUM") as psT, \
             tc.tile_pool(name="psH", bufs=2, space="PSUM") as psH, \
             tc.tile_pool(name="psC", bufs=1, space="PSUM") as psC:

            def allgather(shard, fulls, F, DTY):
                for j in range(BUCK):
                    if LOCAL_SIM:
                        for i in range(QB // 128):
                            tcp = sp.tile([128, F], DTY, tag="agcopy",
                                          name=f"agc_{shard.name}_{j}_{i}")
                            nc.sync.dma_start(
                                out=tcp[:],
                                in_=shard[j * QB + i * 128:j * QB + (i + 1) * 128, :])
                            nc.sync.dma_start(
                                out=fulls[j][i * 128:(i + 1) * 128, :], in_=tcp[:])
                    else:
                        nc.gpsimd.collective_compute(
                            "AllGather", mybir.AluOpType.bypass,
                            replica_groups=[list(range(C))],
                            ins=[shard[j * QB:(j + 1) * QB, :].opt()],
                            outs=[fulls[j][:, :].opt()])

            # constants
            identb = cp.tile([128, 128], BF16)
            make_identity(nc, identb[:])
            identf = cp.tile([128, 128], F32)
            make_identity(nc, identf[:])
            w1sb = cp.tile([128, KI, HID], XDT)
            for k in range(KI):
                nc.sync.dma_start(out=w1sb[:, k, :], in_=w1[k * 128:(k + 1) * 128, :])
            w2sb = cp.tile([128, KH, OUT], XDT)
            for k in range(KH):
                nc.sync.dma_start(out=w2sb[:, k, :], in_=w2[k * 128:(k + 1) * 128, :])
            wbtsb = cp.tile([128, OUT], F32)
            nc.sync.dma_start(out=wbtsb[:], in_=wbt[:, :])
            dvsb = cp.tile([128, NB], F32)
            nc.sync.dma_start(out=dvsb[:], in_=dv_in[:, :])
            dv2sb = cp.tile([128, NB], F32)
            nc.sync.dma_start(out=dv2sb[:], in_=dv2_in[:, :])
            vdsb = cp.tile([128, NB], HDT)
            nc.sync.dma_start(out=vdsb[:], in_=vd_in[:, :])
            b1sb = b2sb = None
            if bias1_nz:
                b1sb = cp.tile([128, F1], F32)
                nc.sync.dma_start(out=b1sb[:], in_=b1_in[:, :])
            if bias2_nz:
                b2sb = cp.tile([128, F2], F32)
                nc.sync.dma_start(out=b2sb[:], in_=b2_in[:, :])
            sc_pos = cp.tile([128, NB], F32, tag="scp")
            sc_neg = cp.tile([128, NB], F32, tag="scn")
            vbc = cp.tile([128, 128], F32, tag="vbc")

            # ---------------- phase A: hw1 = (dinv*x) @ W1 (pos|neg) ---------
            for sb0 in range(0, NB, XSPAN):
                span = min(XSPAN, NB - sb0)
                xp = sp.tile([128, KI, XSPAN * 128], XDT, tag="xtp")
                xn_t = sp.tile([128, KI, XSPAN * 128], XDT, tag="xtn")
                for k in range(KI):
                    nc.sync.dma_start(
                        out=xp[:, k, :span * 128],
                        in_=xtp[k * 128:(k + 1) * 128, sb0 * 128:(sb0 + span) * 128])
                    nc.sync.dma_start(
                        out=xn_t[:, k, :span * 128],
                        in_=xtn[k * 128:(k + 1) * 128, sb0 * 128:(sb0 + span) * 128])
                for j in range(span):
                    nb_ = sb0 + j
                    pa = psA.tile([128, F1], F32, tag="agg", space="PSUM")
                    for k in range(KI):
                        nc.tensor.matmul(
                            out=pa[:, 0:HID],
                            lhsT=xp[:, k, j * 128:(j + 1) * 128],
                            rhs=w1sb[:, k, :],
                            start=(k == 0), stop=(k == KI - 1))
                    for k in range(KI):
                        nc.tensor.matmul(
                            out=pa[:, HID:F1],
                            lhsT=xn_t[:, k, j * 128:(j + 1) * 128],
                            rhs=w1sb[:, k, :],
                            start=(k == 0), stop=(k == KI - 1))
                    hw1sb = sp.tile([128, F1], TDT1, tag="hw1sb")
                    nc.scalar.activation(out=hw1sb[:], in_=pa[:], func=ACT.Copy)
                    nc.sync.dma_start(out=hw1t_sh[nb_ * 128:(nb_ + 1) * 128, :],
                                      in_=hw1sb[:])

            allgather(hw1t_sh, hw1_full, F1, TDT1)

            # ---------------- aggregation layers ----------------
            def agg_layer(layer):
                F = F1 if layer == 1 else F2
                DTY = TDT1 if layer == 1 else GDT2
                fulls = hw1_full if layer == 1 else hw2_full
                bsb = b1sb if layer == 1 else b2sb
                use_dr = DR
                for mgi in range(NB // MG):
                    b0 = mgi * MG
                    mg0 = int(chunk_off[b0, 0])
                    gcols = mg_cols[mgi]
                    mt = mpp.tile([128, MGC, 128], F8E3, tag="mt",
                                  name=f"mt{layer}_{b0}")
                    ix = mpp.tile([128, MGC * 8], I16, tag="ix",
                                  name=f"ix{layer}_{b0}")
                    nc.scalar.dma_start(
                        out=mt[:, :gcols, :],
                        in_=m_in[:, mg0 * 128:(mg0 + gcols) * 128])
                    nc.scalar.dma_start(
                        out=ix[:, :gcols * 8],
                        in_=idx_in[:, mg0 * 8:(mg0 + gcols) * 8])
                    # gathers per (GSUB-block sub-group, bucket)
                    gts = {}   # (gg, u) -> tile
                    if ABLATE != "nogather":
                        for gg in range(b0 // GSUB, (b0 + MG) // GSUB):
                            gb0 = gg * GSUB
                            for u in range(BUCK):
                                runs = gg_runs[gg][u]
                                if not runs:
                                    continue
                                co = int(chunk_off[gb0, u])
                                gtv = gp.tile([128, nmax, F], DTY, tag="gat",
                                              name=f"gt{layer}_{gg}_{u}")
                                for (rs, rn) in runs:
                                    nc.gpsimd.dma_gather(
                                        out_ap=gtv[:, rs:rs + rn, :],
                                        in_ap=fulls[u][:, :],
                                        idxs_ap=ix[:, (co - mg0 + rs) * 8:
                                                   (co - mg0 + rs + rn) * 8],
                                        num_idxs=rn * 128,
                                        num_idxs_reg=rn * 128,
                                        elem_size=F,
                                        single_packet=(
                                            True if SP_MODE == 1 else
                                            False if SP_MODE == 2 else
                                            rn * 128 <= 1024),
                                        queue_num=u)
                                gts[(gg, u)] = gtv
                    for b in range(b0, b0 + MG):
                        ggb = b // GSUB
                        kb = 0 if ABLATE == "nogather" else int(K[b].sum())
                        ps_agg = psA.tile([128, F1], F32, tag="agg", space="PSUM")
                        if kb > 0 and ABLATE != "nomm":
                            # count instructions for start/stop flags
                            insts = []
                            for u in range(BUCK):
                                ku = int(K[b, u])
                                if ku == 0:
                                    continue
                                lo = int(chunk_off[b, u]) - mg0
                                go = (int(chunk_off[b, u])
                                      - int(chunk_off[ggb * GSUB, u]))
                                j = 0
                                while j < ku:
                                    step = 2 if (use_dr and j + 2 <= ku) else 1
                                    insts.append((u, lo + j, go + j, step))
                                    j += step
                            for t, (u, lo, go, step) in enumerate(insts):
                                if step == 2:
                                    nc.tensor.matmul(
                                        out=ps_agg[:, :F],
                                        lhsT=mt[:, lo:lo + 2, :],
                                        rhs=gts[(ggb, u)][:, go:go + 2, :],
                                        perf_mode=mybir.MatmulPerfMode.DoubleRow,
                                        start=(t == 0), stop=(t == len(insts) - 1))
                                else:
                                    nc.tensor.matmul(
                                        out=ps_agg[:, :F],
                                        lhsT=mt[:, lo, :],
                                        rhs=gts[(ggb, u)][:, go, :],
                                        start=(t == 0), stop=(t == len(insts) - 1))
                        else:
                            nc.vector.memset(ps_agg[:], 0.0)

                        if layer == 1:
                            hout = sp.tile([128, F1], XDT, tag="hout",
                                           name=f"ho1_{b}")
                            if bsb is None:
                                nc.scalar.activation(out=hout[:], in_=ps_agg[:],
                                                     func=ACT.Relu)
                            else:
                                nc.scalar.activation(
                                    out=hout[:], in_=ps_agg[:], func=ACT.Copy,
                                    scale=dvsb[:, b:b + 1])
                                nc.vector.tensor_tensor(
                                    out=hout[:], in0=hout[:], in1=bsb[:],
                                    op=mybir.AluOpType.add)
                                nc.vector.tensor_scalar(
                                    out=hout[:], in0=hout[:], scalar1=0.0,
                                    scalar2=None, op0=mybir.AluOpType.max)
                            ps_tp = psT.tile([128, F1], XDT, tag="tp",
                                             space="PSUM", name=f"tp1_{b}")
                            for k in range(2 * KH):
                                nc.tensor.transpose(
                                    out=ps_tp[:, k * 128:(k + 1) * 128],
                                    in_=hout[:, k * 128:(k + 1) * 128],
                                    identity=identb[:])
                            ts = sp.tile([128, F1], XDT, tag="ts", name=f"ts1_{b}")
                            nc.scalar.copy(out=ts[:], in_=ps_tp[:])
                            ps_h2 = psH.tile([128, F2], F32, tag="h2",
                                             space="PSUM", name=f"h2_{b}")
                            for k in range(KH):
                                nc.tensor.matmul(
                                    out=ps_h2[:, 0:OUT],
                                    lhsT=ts[:, k * 128:(k + 1) * 128],
                                    rhs=w2sb[:, k, :],
                                    start=(k == 0), stop=(k == KH - 1))
                            for k in range(KH):
                                nc.tensor.matmul(
                                    out=ps_h2[:, OUT:F2],
                                    lhsT=ts[:, (KH + k) * 128:(KH + k + 1) * 128],
                                    rhs=w2sb[:, k, :],
                                    start=(k == 0), stop=(k == KH - 1))
                            # layer-2 table rows: dinv^2 * (relu(s1) @ W2)
                            # (postponed layer-1 dinv_dst and layer-2 src factor)
                            hw2sb = sp.tile([128, F2], GDT2, tag="hw2sb",
                                            name=f"hw2sb_{b}")
                            nc.scalar.activation(
                                out=hw2sb[:], in_=ps_h2[:], func=ACT.Copy,
                                scale=dv2sb[:, b:b + 1] if bsb is None
                                else dvsb[:, b:b + 1])
                            nc.sync.dma_start(
                                out=hw2_sh[b * 128:(b + 1) * 128, :],
                                in_=hw2sb[:])
                        else:
                            hout = sp.tile([128, F2], HDT, tag="hout2",
                                           name=f"ho2_{b}")
                            nc.scalar.activation(
                                out=hout[:], in_=ps_agg[:, :F2], func=ACT.Copy,
                                scale=dvsb[:, b:b + 1])
                            if bsb is not None:
                                nc.vector.tensor_tensor(
                                    out=hout[:], in0=hout[:], in1=bsb[:],
                                    op=mybir.AluOpType.add)
                            if b == 0:
                                ps_cs = psC.tile([128, 1], F32, tag="cs",
                                                 space="PSUM")
                                agg_layer.cs = ps_cs
                            else:
                                ps_cs = agg_layer.cs
                            nc.tensor.matmul(
                                out=ps_cs[:], lhsT=hout[:, 0:OUT],
                                rhs=vdsb[:, b:b + 1],
                                start=(b == 0), stop=(b == NB - 1),
                                skip_group_check=True)
                            nc.sync.dma_start(
                                out=h2d[:, b * F2:(b + 1) * F2], in_=hout[:])

            agg_layer(1)
            allgather(hw2_sh, hw2_full, F2, GDT2)
            agg_layer(2)

            # ---------------- summary s and v = Wb @ s ----------------
            cssb = sp.tile([128, 1], F32, tag="cssb")
            nc.scalar.copy(out=cssb[:], in_=agg_layer.cs[:])
            nc.sync.dma_start(out=cs_in[:, :], in_=cssb[:])
            if LOCAL_SIM:
                nc.sync.dma_start(out=cs_out[:, :], in_=cssb[:])
            else:
                nc.gpsimd.collective_compute(
                    "AllReduce", mybir.AluOpType.add,
                    replica_groups=[list(range(C))],
                    ins=[cs_in[:, :].opt()], outs=[cs_out[:, :].opt()])
            csr = sp.tile([128, 1], F32, tag="csr")
            nc.sync.dma_start(out=csr[:], in_=cs_out[:, :])
            ssb = sp.tile([128, 1], F32, tag="ssb")
            nc.scalar.activation(out=ssb[:], in_=csr[:], func=ACT.Sigmoid,
                                 scale=1.0 / N)
            ps_v = psC.tile([128, 1], F32, tag="cs", space="PSUM")
            nc.tensor.matmul(out=ps_v[:], lhsT=wbtsb[:], rhs=ssb[:],
                             start=True, stop=True)
            vsb = sp.tile([128, 1], F32, tag="vsb")
            nc.scalar.copy(out=vsb[:], in_=ps_v[:])
            # v^T broadcast to all partitions: vbc[p, f] = v[f]
            ps_vt = psH.tile([128, F2], F32, tag="h2", space="PSUM")
            nc.tensor.transpose(out=ps_vt[0:1, 0:128], in_=vsb[:, 0:1],
                                identity=identf[:])
            vrow = sp.tile([128, 128], F32, tag="vrow")
            nc.scalar.copy(out=vrow[0:1, :], in_=ps_vt[0:1, 0:128])
            nc.gpsimd.partition_broadcast(out_ap=vbc[:], in_ap=vrow[0:1, :])

            # ---------------- scores: sc[p, b] = H[p, b, :] . v ----------
            SCB = 8
            for sb0 in range(0, NB, SCB):
                span = min(SCB, NB - sb0)
                hld = sp.tile([128, SCB * F2], HDT, tag="hld")
                nc.sync.dma_start(out=hld[:, :span * F2],
                                  in_=h2d[:, sb0 * F2:(sb0 + span) * F2])
                for j in range(span):
                    b = sb0 + j
                    prod = sp.tile([128, F2], F32, tag="prod")
                    nc.vector.tensor_tensor(
                        out=prod[:, 0:128], in0=hld[:, j * F2:j * F2 + 128],
                        in1=vbc[:], op=mybir.AluOpType.mult)
                    nc.vector.tensor_tensor(
                        out=prod[:, 128:F2],
                        in0=hld[:, j * F2 + 128:(j + 1) * F2],
                        in1=vbc[:], op=mybir.AluOpType.mult)
                    nc.vector.reduce_sum(out=sc_pos[:, b:b + 1],
                                         in_=prod[:, 0:128],
                                         axis=mybir.AxisListType.X)
                    nc.vector.reduce_sum(out=sc_neg[:, b:b + 1],
                                         in_=prod[:, 128:F2],
                                         axis=mybir.AxisListType.X)
            if bb_val != 0.0:
                nc.scalar.activation(out=sc_pos[:], in_=sc_pos[:],
                                     func=ACT.Copy, bias=float(bb_val))
                nc.scalar.activation(out=sc_neg[:], in_=sc_neg[:],
                                     func=ACT.Copy, bias=float(bb_val))
            nc.sync.dma_start(out=out[0, :, :], in_=sc_pos[:])
            nc.sync.dma_start(out=out[1, :, :], in_=sc_neg[:])

    nc.compile()
    return nc


# ----------------------------------------------------------------------------
# entry point
# ----------------------------------------------------------------------------

_CACHE = {}


def _get_program(meta, HID, OUT, bias1_nz, bias2_nz, bb_val, C):
    key = (meta["N"], meta["E"], meta["IN"], HID, OUT, bias1_nz, bias2_nz,
           float(bb_val), C, meta["TOT"], meta["K"].tobytes())
    if key not in _CACHE:
        _CACHE[key] = _build(meta, HID, OUT, bias1_nz, bias2_nz, bb_val, C)
    return _CACHE[key]


def _make_in_maps(meta, arrs, W1, b1, W2, b2, Wb, C, bias1_nz, bias2_nz):
    wdt = mybir.dt.np(BF16)
    W1, W2 = W1.astype(wdt), W2.astype(wdt)
    in_maps = []
    for c in range(C):
        m = {
            "xtp": arrs["xT_pos"][c], "xtn": arrs["xT_neg"][c],
            "w1": W1, "w2": W2, "wbt": np.ascontiguousarray(Wb.T),
            "idx16": arrs["idx_dev"][c], "monehot": arrs["m_dev"][c],
            "dinv": arrs["dv_dev"][c], "dinv2": arrs["dv2_dev"][c],
            "valid": arrs["vd_dev"][c],
        }
        if bias1_nz:
            m["b1bc"] = np.tile(np.concatenate([b1, b1])[None, :], (128, 1))
        if bias2_nz:
            m["b2bc"] = np.tile(np.concatenate([b2, b2])[None, :], (128, 1))
        in_maps.append(m)
    return in_maps


def _unpack(meta, results):
    N = meta["N"]
    core, blk, slot = meta["core"], meta["blk"], meta["slot"]
    pos = np.empty((N, 1), np.float32)
    neg = np.empty((N, 1), np.float32)
    sc0 = np.stack([np.asarray(results[c]["scores"][0]) for c in range(len(results))])
    sc1 = np.stack([np.asarray(results[c]["scores"][1]) for c in range(len(results))])
    pos[:, 0] = sc0[core, slot, blk]
    neg[:, 0] = sc1[core, slot, blk]
    return pos, neg


def kernel(x, edge_index, perm, W1, b1, W2, b2, Wb, bb):
    C = 8
    x = np.asarray(x, np.float32)
    W1 = np.asarray(W1, np.float32)
    W2 = np.asarray(W2, np.float32)
    Wb = np.asarray(Wb, np.float32)
    b1 = np.asarray(b1, np.float32)
    b2 = np.asarray(b2, np.float32)
    bb_val = float(np.asarray(bb).reshape(-1)[0])
    HID = W1.shape[1]
    OUT = W2.shape[1]

    meta, arrs = _prep(x, edge_index, perm, C)
    bias1_nz = bool(np.any(b1))
    bias2_nz = bool(np.any(b2))
    nc = _get_program(meta, HID, OUT, bias1_nz, bias2_nz, bb_val, C)
    in_maps = _make_in_maps(meta, arrs, W1, b1, W2, b2, Wb, C, bias1_nz, bias2_nz)

    res = bass_utils.run_bass_kernel_spmd(nc, in_maps, core_ids=list(range(C)))
    return _unpack(meta, res.results)


# revision 18
# speedup vs baseline: 1.3337x; 1.0003x over previous
"""DGI (2-layer GCN encoder + bilinear discriminator) on 8 TRN2 NeuronCores.

v2 design. Nodes are assigned to (core, block, slot) positions with a
degree-balanced round-robin deal (high in-degree nodes dealt first across the
8*NB bins), which keeps per-(block, bucket) edge counts near-uniform so the
128-edge chunk count K is ~4 everywhere. Self-loops are real edges, so there
is no separate self-row path. Per layer, each core computes its shard of
h @ W (pos|neg fused on the feature axis), the full feature table is
AllGathered in 4 bucket slices (<=32K rows each for int16 gather indices),
then the segment-sum runs as dma_gather row fetches x one-hot selection
matrices accumulated in PSUM by the PE. Gathers are grouped per (block-pair,
bucket) to amortize the ~1us fixed SWDGE descriptor-generation cost; padded
slots index row 0 (their one-hot rows are zero), so no truncation registers
or warm-up passes are needed. The layer-1 table is fp8 e4m3 and the one-hot
matrices are e4m3 too, so layer-1 aggregation matmuls run in DoubleRow perf
mode (2 chunks / instruction at 0.5 cyc/row); layer-2 stays bf16 (e4m3 there
pushes quantization error past the error budget). The GCN edge norm is
factorized into the tables: x rows are pre-scaled by dinv on the host, the
layer-2 table rows are scaled by dinv^2 (postponing layer-1's dinv_dst scale
through the relu and W2), and each layer's output is post-scaled by dinv on
the Activation engine, which also runs all other per-block post ops (relu,
casts, score bias) to keep DVE and the DMA-heavy engines free. The
transposed H needed by the bilinear scores is cached in SBUF during layer-2
so the score phase is just two matvecs per block.
"""
import sys
sys.path.insert(0, "/opt/trn_rl_repo")

import numpy as np
import concourse.bass as bass
import concourse.bacc as bacc
import concourse.tile as tile
from concourse import bass_utils, mybir
from concourse.masks import make_identity

F32 = mybir.dt.float32
BF16 = mybir.dt.bfloat16
I16 = mybir.dt.int16
F8E4 = mybir.dt.float8e4
F8E3 = mybir.dt.float8e3

BUCK = 4             # table buckets (each <= 32K rows for int16 indices)
MG = 4               # blocks per meta (M/idx) load group
NCAP = 20            # max 128-row chunks per dma_gather (desc-ring headroom)
GSUB = 2             # blocks per gather group (divides MG)
SP_MODE = 0          # single_packet: 0=auto (<=1024 idxs), 1=always, 2=never
MPP_BUFS = 3
GAT_BUFS = 8
SP_BUFS = 3
PSA_BUFS = 3
XSPAN = 4
DR = False           # DoubleRow fp8 matmuls (requires e4m3 tables)
L2_F8 = True         # layer-2 table in fp8 (upscaled e3m4)
UP2 = 16.0           # layer-2 table upscale (folded out of dinv post-scale)
LOCAL_SIM = False    # replace collectives with local copies (TimelineSim)
ABLATE = ""          # "", "nogather" (skip gathers+matmuls), "nomm" (skip matmuls)


def _cdiv(a, b):
    return -(-a // b)


# ----------------------------------------------------------------------------
# host-side preprocessing
# ----------------------------------------------------------------------------

def _prep(x, edge_index, perm, C):
    N, IN = x.shape
    E = edge_index.shape[1]
    assert N % C == 0
    SH = N // C
    # ~14% slot slack so balanced (block, bucket) edge counts stay under the
    # 4-chunk (512-edge) ceiling; NB multiple of 4 aligns buckets to blocks.
    NB = _cdiv(_cdiv(SH, 128) * 8, 7)
    NB = _cdiv(NB, 4) * 4
    NP = NB * 128
    QB = NP // BUCK
    BS = C * QB
    assert BS <= 32704

    src = np.asarray(edge_index[0], dtype=np.int64)
    dst = np.asarray(edge_index[1], dtype=np.int64)
    perm = np.asarray(perm, dtype=np.int64)

    deg = (1.0 + np.bincount(dst, minlength=N)).astype(np.float32)
    dinv = (1.0 / np.sqrt(deg)).astype(np.float32)

    # node -> (core, block, slot): deal nodes in descending-degree order
    # round-robin across all C*NB bins (cores fastest) to balance both core
    # and block load.
    order = np.argsort(-deg, kind="stable")
    nbins = C * NB
    i = np.arange(N, dtype=np.int64)
    core = np.empty(N, np.int64)
    blk = np.empty(N, np.int64)
    slot = np.empty(N, np.int64)
    core[order] = i % C
    blk[order] = (i // C) % NB
    slot[order] = i // nbins
    assert slot.max() < 128
    sl = blk * 128 + slot                   # local row within the core

    # self-loops as edges
    srcA = np.concatenate([src, np.arange(N, dtype=np.int64)])
    dstA = np.concatenate([dst, np.arange(N, dtype=np.int64)])
    EA = srcA.shape[0]

    dc = core[dstA]
    db = blk[dstA]
    dloc = slot[dstA]
    ssl = sl[srcA]
    su = ssl // QB
    srow = (core[srcA] * QB + (ssl - su * QB)).astype(np.int64)

    cnt = np.bincount((dc * NB + db) * BUCK + su,
                      minlength=C * NB * BUCK).reshape(C, NB, BUCK)
    K = _cdiv(cnt, 128).max(axis=0)         # [NB, BUCK]

    # chunk layout ordered by (meta group mg=b//MG, bucket u, block b, chunk
    # k) so a group's bucket-u chunks are contiguous for one grouped gather.
    assert NB % MG == 0
    chunk_off = np.zeros((NB, BUCK), np.int64)
    tot = 0
    for mg in range(NB // MG):
        for u in range(BUCK):
            for b in range(mg * MG, (mg + 1) * MG):
                chunk_off[b, u] = tot
                tot += int(K[b, u])
    TOT = tot

    fgrp = (dc * NB + db) * BUCK + su
    order_e = np.argsort(fgrp, kind="stable")
    fgrp_s = fgrp[order_e]
    gstart = np.concatenate([[0], np.cumsum(np.bincount(fgrp, minlength=C * NB * BUCK))])
    rank = np.arange(EA, dtype=np.int64) - gstart[fgrp_s]
    bu_s = fgrp_s % (NB * BUCK)
    slot_e = chunk_off.reshape(-1)[bu_s] * 128 + rank
    dc_s = fgrp_s // (NB * BUCK)

    idxbuf = np.zeros((C, TOT * 128), np.int16)   # pads index row 0
    idxbuf[dc_s, slot_e] = srow[order_e].astype(np.int16)
    mdt = mybir.dt.np(F8E3)
    mbuf = np.zeros((C, TOT * 128, 128), mdt)
    mbuf[dc_s, slot_e, dloc[order_e]] = mdt(1.0)  # pads stay zero rows

    idx_dev = np.tile(
        idxbuf.reshape(C, TOT, 8, 16).transpose(0, 3, 1, 2).reshape(C, 16, TOT * 8),
        (1, 8, 1),
    )  # [C, 128, TOT*8]
    m_dev = np.ascontiguousarray(
        mbuf.reshape(C, TOT, 128, 128).transpose(0, 2, 1, 3)
        .reshape(C, 128, TOT * 128))

    # per-position dinv / dinv^2 / valid, [C, 128, NB] (partition = slot)
    dvfull = np.zeros(C * NP, np.float32)
    dvfull[core * NP + sl] = dinv
    dv_raw = dvfull.reshape(C, NB, 128).transpose(0, 2, 1).copy()
    dv2_dev = (dv_raw * dv_raw) * UP2     # layer-2 table write scale
    dv_dev = dv_raw / UP2                 # layer-2 output scale
    vdfull = np.zeros(C * NP, np.float32)
    vdfull[core * NP + sl] = 1.0
    vd_dev = vdfull.reshape(C, NB, 128).transpose(0, 2, 1).astype(mybir.dt.np(BF16))

    # x tables, dinv pre-folded, bf16, feature-major [C, IN, NP]
    xdt = mybir.dt.np(BF16)
    xs = (x * dinv[:, None])
    xn = (x[perm] * dinv[:, None])
    xT_pos = np.zeros((C, IN, NP), xdt)
    xT_neg = np.zeros((C, IN, NP), xdt)
    xT_pos[core, :, sl] = xs.astype(xdt)
    xT_neg[core, :, sl] = xn.astype(xdt)

    meta = dict(N=N, E=E, IN=IN, SH=SH, NB=NB, NP=NP, QB=QB, BS=BS, TOT=TOT,
                K=K, chunk_off=chunk_off, core=core, blk=blk, slot=slot)
    arrays = dict(idx_dev=idx_dev, m_dev=m_dev, dv_dev=dv_dev,
                  dv2_dev=dv2_dev, vd_dev=vd_dev, xT_pos=xT_pos, xT_neg=xT_neg)
    return meta, arrays


# ----------------------------------------------------------------------------
# device program
# ----------------------------------------------------------------------------

def _build(meta, HID, OUT, bias1_nz, bias2_nz, bb_val, C):
    N, IN = meta["N"], meta["IN"]
    NB, NP, QB, BS = meta["NB"], meta["NP"], meta["QB"], meta["BS"]
    TOT = meta["TOT"]
    K, chunk_off = meta["K"], meta["chunk_off"]
    KI, KH = IN // 128, HID // 128
    assert OUT == 128, "discriminator path assumes OUT == 128"
    F1, F2 = 2 * HID, 2 * OUT
    TDT1 = F8E3          # layer-1 table dtype
    GDT2 = F8E3 if L2_F8 else BF16   # layer-2 table dtype
    XDT = BF16
    HDT = BF16

    # gather runs per (gather group of GSUB blocks, bucket): contiguous
    # chunk spans of <= NCAP, offsets relative to the gather group start
    assert MG % GSUB == 0
    NG = NB // GSUB
    gg_runs = [[None] * BUCK for _ in range(NG)]
    nmax = 1
    for gg in range(NG):
        bs = range(gg * GSUB, (gg + 1) * GSUB)
        for u in range(BUCK):
            n = sum(int(K[b, u]) for b in bs)
            runs = []
            s = 0
            while s < n:
                rn = min(NCAP, n - s)
                runs.append((s, rn))
                s += rn
            gg_runs[gg][u] = runs
            nmax = max(nmax, n)
    # meta (M/idx) group column extents
    mg_cols = []
    for b0 in range(0, NB, MG):
        be = min(NB, b0 + MG)
        c0 = int(chunk_off[b0, 0])
        c1 = TOT if be == NB else int(chunk_off[be, 0])
        mg_cols.append(c1 - c0)
    MGC = max(mg_cols)

    nc = bacc.Bacc("TRN2", target_bir_lowering=False, debug=False, num_devices=C,
                   num_swdge_queues=4, dynamic_dma_scratch_size=24576)

    # inputs
    xtp = nc.dram_tensor("xtp", [IN, NP], XDT, kind="ExternalInput")
    xtn = nc.dram_tensor("xtn", [IN, NP], XDT, kind="ExternalInput")
    w1 = nc.dram_tensor("w1", [IN, HID], XDT, kind="ExternalInput")
    w2 = nc.dram_tensor("w2", [HID, OUT], XDT, kind="ExternalInput")
    wbt = nc.dram_tensor("wbt", [OUT, OUT], F32, kind="ExternalInput")
    idx_in = nc.dram_tensor("idx16", [128, TOT * 8], I16, kind="ExternalInput")
    m_in = nc.dram_tensor("monehot", [128, TOT * 128], F8E3, kind="ExternalInput")
    dv_in = nc.dram_tensor("dinv", [128, NB], F32, kind="ExternalInput")
    dv2_in = nc.dram_tensor("dinv2", [128, NB], F32, kind="ExternalInput")
    vd_in = nc.dram_tensor("valid", [128, NB], HDT, kind="ExternalInput")
    b1_in = nc.dram_tensor("b1bc", [128, F1], F32, kind="ExternalInput") if bias1_nz else None
    b2_in = nc.dram_tensor("b2bc", [128, F2], F32, kind="ExternalInput") if bias2_nz else None
    out = nc.dram_tensor("scores", [2, 128, NB], F32, kind="ExternalOutput")

    # internal DRAM
    hw1t_sh = nc.dram_tensor("hw1t_sh", [NP, F1], TDT1, kind="Internal")
    hw2_sh = nc.dram_tensor("hw2_sh", [NP, F2], GDT2, kind="Internal")
    hw1_full = [nc.dram_tensor(f"hw1_full{j}", [BS, F1], TDT1, kind="Internal",
                               addr_space="Shared") for j in range(BUCK)]
    hw2_full = [nc.dram_tensor(f"hw2_full{j}", [BS, F2], GDT2, kind="Internal",
                               addr_space="Shared") for j in range(BUCK)]
    h2d = nc.dram_tensor("h2d", [128, NB * F2], GDT2 if False else BF16,
                         kind="Internal")
    cs_in = nc.dram_tensor("cs_in", [128, 1], F32, kind="Internal")
    cs_out = nc.dram_tensor("cs_out", [128, 1], F32, kind="Internal",
                            addr_space="Shared")

    ACT = mybir.ActivationFunctionType

    with tile.TileContext(nc) as tc:
        with tc.tile_pool(name="const", bufs=1) as cp, \
             tc.tile_pool(name="stream", bufs=SP_BUFS) as sp, \
             tc.tile_pool(name="meta", bufs=MPP_BUFS) as mpp, \
             tc.tile_pool(name="gat", bufs=GAT_BUFS) as gp, \
             tc.tile_pool(name="psA", bufs=PSA_BUFS, space="PSUM") as psA, \
             tc.tile_pool(name="psT", bufs=2, space="PSUM") as psT, \
             tc.tile_pool(name="psH", bufs=2, space="PSUM") as psH, \
             tc.tile_pool(name="psC", bufs=1, space="PSUM") as psC:

            def allgather(shard, fulls, F, DTY):
                for j in range(BUCK):
                    if LOCAL_SIM:
                        for i in range(QB // 128):
                            tcp = sp.tile([128, F], DTY, tag="agcopy",
                                          name=f"agc_{shard.name}_{j}_{i}")
                            nc.sync.dma_start(
                                out=tcp[:],
                                in_=shard[j * QB + i * 128:j * QB + (i + 1) * 128, :])
                            nc.sync.dma_start(
                                out=fulls[j][i * 128:(i + 1) * 128, :], in_=tcp[:])
                    else:
                        nc.gpsimd.collective_compute(
                            "AllGather", mybir.AluOpType.bypass,
                            replica_groups=[list(range(C))],
                            ins=[shard[j * QB:(j + 1) * QB, :].opt()],
                            outs=[fulls[j][:, :].opt()])

            # constants
            identb = cp.tile([128, 128], BF16)
            make_identity(nc, identb[:])
            identf = cp.tile([128, 128], F32)
            make_identity(nc, identf[:])
            w1sb = cp.tile([128, KI, HID], XDT)
            for k in range(KI):
                nc.sync.dma_start(out=w1sb[:, k, :], in_=w1[k * 128:(k + 1) * 128, :])
            w2sb = cp.tile([128, KH, OUT], XDT)
            for k in range(KH):
                nc.sync.dma_start(out=w2sb[:, k, :], in_=w2[k * 128:(k + 1) * 128, :])
            wbtsb = cp.tile([128, OUT], F32)
            nc.sync.dma_start(out=wbtsb[:], in_=wbt[:, :])
            dvsb = cp.tile([128, NB], F32)
            nc.sync.dma_start(out=dvsb[:], in_=dv_in[:, :])
            dv2sb = cp.tile([128, NB], F32)
            nc.sync.dma_start(out=dv2sb[:], in_=dv2_in[:, :])
            vdsb = cp.tile([128, NB], HDT)
            nc.sync.dma_start(out=vdsb[:], in_=vd_in[:, :])
            b1sb = b2sb = None
            if bias1_nz:
                b1sb = cp.tile([128, F1], F32)
                nc.sync.dma_start(out=b1sb[:], in_=b1_in[:, :])
            if bias2_nz:
                b2sb = cp.tile([128, F2], F32)
                nc.sync.dma_start(out=b2sb[:], in_=b2_in[:, :])
            sc_pos = cp.tile([128, NB], F32, tag="scp")
            sc_neg = cp.tile([128, NB], F32, tag="scn")
            vbc = cp.tile([128, 128], F32, tag="vbc")

            # ---------------- phase A: hw1 = (dinv*x) @ W1 (pos|neg) ---------
            for sb0 in range(0, NB, XSPAN):
                span = min(XSPAN, NB - sb0)
                xp = sp.tile([128, KI, XSPAN * 128], XDT, tag="xtp")
                xn_t = sp.tile([128, KI, XSPAN * 128], XDT, tag="xtn")
                for k in range(KI):
                    nc.sync.dma_start(
                        out=xp[:, k, :span * 128],
                        in_=xtp[k * 128:(k + 1) * 128, sb0 * 128:(sb0 + span) * 128])
                    nc.sync.dma_start(
                        out=xn_t[:, k, :span * 128],
                        in_=xtn[k * 128:(k + 1) * 128, sb0 * 128:(sb0 + span) * 128])
                for j in range(span):
                    nb_ = sb0 + j
                    pa = psA.tile([128, F1], F32, tag="agg", space="PSUM")
                    for k in range(KI):
                        nc.tensor.matmul(
                            out=pa[:, 0:HID],
                            lhsT=xp[:, k, j * 128:(j + 1) * 128],
                            rhs=w1sb[:, k, :],
                            start=(k == 0), stop=(k == KI - 1))
                    for k in range(KI):
                        nc.tensor.matmul(
                            out=pa[:, HID:F1],
                            lhsT=xn_t[:, k, j * 128:(j + 1) * 128],
                            rhs=w1sb[:, k, :],
                            start=(k == 0), stop=(k == KI - 1))
                    hw1sb = sp.tile([128, F1], TDT1, tag="hw1sb")
                    nc.scalar.activation(out=hw1sb[:], in_=pa[:], func=ACT.Copy)
                    nc.sync.dma_start(out=hw1t_sh[nb_ * 128:(nb_ + 1) * 128, :],
                                      in_=hw1sb[:])

            allgather(hw1t_sh, hw1_full, F1, TDT1)

            # ---------------- aggregation layers ----------------
            def agg_layer(layer):
                F = F1 if layer == 1 else F2
                DTY = TDT1 if layer == 1 else GDT2
                fulls = hw1_full if layer == 1 else hw2_full
                bsb = b1sb if layer == 1 else b2sb
                use_dr = DR
                for mgi in range(NB // MG):
                    b0 = mgi * MG
                    mg0 = int(chunk_off[b0, 0])
                    gcols = mg_cols[mgi]
                    mt = mpp.tile([128, MGC, 128], F8E3, tag="mt",
                                  name=f"mt{layer}_{b0}")
                    ix = mpp.tile([128, MGC * 8], I16, tag="ix",
                                  name=f"ix{layer}_{b0}")
                    nc.scalar.dma_start(
                        out=mt[:, :gcols, :],
                        in_=m_in[:, mg0 * 128:(mg0 + gcols) * 128])
                    nc.scalar.dma_start(
                        out=ix[:, :gcols * 8],
                        in_=idx_in[:, mg0 * 8:(mg0 + gcols) * 8])
                    # gathers per (GSUB-block sub-group, bucket)
                    gts = {}   # (gg, u) -> tile
                    if ABLATE != "nogather":
                        for gg in range(b0 // GSUB, (b0 + MG) // GSUB):
                            gb0 = gg * GSUB
                            for u in range(BUCK):
                                runs = gg_runs[gg][u]
                                if not runs:
                                    continue
                                co = int(chunk_off[gb0, u])
                                gtv = gp.tile([128, nmax, F], DTY, tag="gat",
                                              name=f"gt{layer}_{gg}_{u}")
                                for (rs, rn) in runs:
                                    nc.gpsimd.dma_gather(
                                        out_ap=gtv[:, rs:rs + rn, :],
                                        in_ap=fulls[u][:, :],
                                        idxs_ap=ix[:, (co - mg0 + rs) * 8:
                                                   (co - mg0 + rs + rn) * 8],
                                        num_idxs=rn * 128,
                                        num_idxs_reg=rn * 128,
                                        elem_size=F,
                                        single_packet=(
                                            True if SP_MODE == 1 else
                                            False if SP_MODE == 2 else
                                            rn * 128 <= 1024),
                                        queue_num=u)
                                gts[(gg, u)] = gtv
                    for b in range(b0, b0 + MG):
                        ggb = b // GSUB
                        kb = 0 if ABLATE == "nogather" else int(K[b].sum())
                        ps_agg = psA.tile([128, F1], F32, tag="agg", space="PSUM")
                        if kb > 0 and ABLATE != "nomm":
                            # count instructions for start/stop flags
                            insts = []
                            for u in range(BUCK):
                                ku = int(K[b, u])
                                if ku == 0:
                                    continue
                                lo = int(chunk_off[b, u]) - mg0
                                go = (int(chunk_off[b, u])
                                      - int(chunk_off[ggb * GSUB, u]))
                                j = 0
                                while j < ku:
                                    step = 2 if (use_dr and j + 2 <= ku) else 1
                                    insts.append((u, lo + j, go + j, step))
                                    j += step
                            for t, (u, lo, go, step) in enumerate(insts):
                                if step == 2:
                                    nc.tensor.matmul(
                                        out=ps_agg[:, :F],
                                        lhsT=mt[:, lo:lo + 2, :],
                                        rhs=gts[(ggb, u)][:, go:go + 2, :],
                                        perf_mode=mybir.MatmulPerfMode.DoubleRow,
                                        start=(t == 0), stop=(t == len(insts) - 1))
                                else:
                                    nc.tensor.matmul(
                                        out=ps_agg[:, :F],
                                        lhsT=mt[:, lo, :],
                                        rhs=gts[(ggb, u)][:, go, :],
                                        start=(t == 0), stop=(t == len(insts) - 1))
                        else:
                            nc.vector.memset(ps_agg[:], 0.0)

                        if layer == 1:
                            hout = sp.tile([128, F1], XDT, tag="hout",
                                           name=f"ho1_{b}")
                            if bsb is None:
                                nc.scalar.activation(out=hout[:], in_=ps_agg[:],
                                                     func=ACT.Relu)
                            else:
                                nc.scalar.activation(
                                    out=hout[:], in_=ps_agg[:], func=ACT.Copy,
                                    scale=dvsb[:, b:b + 1])
                                nc.vector.tensor_tensor(
                                    out=hout[:], in0=hout[:], in1=bsb[:],
                                    op=mybir.AluOpType.add)
                                nc.vector.tensor_scalar(
                                    out=hout[:], in0=hout[:], scalar1=0.0,
                                    scalar2=None, op0=mybir.AluOpType.max)
                            ps_tp = psT.tile([128, F1], XDT, tag="tp",
                                             space="PSUM", name=f"tp1_{b}")
                            for k in range(2 * KH):
                                nc.tensor.transpose(
                                    out=ps_tp[:, k * 128:(k + 1) * 128],
                                    in_=hout[:, k * 128:(k + 1) * 128],
                                    identity=identb[:])
                            ts = sp.tile([128, F1], XDT, tag="ts", name=f"ts1_{b}")
                            nc.scalar.copy(out=ts[:], in_=ps_tp[:])
                            ps_h2 = psH.tile([128, F2], F32, tag="h2",
                                             space="PSUM", name=f"h2_{b}")
                            for k in range(KH):
                                nc.tensor.matmul(
                                    out=ps_h2[:, 0:OUT],
                                    lhsT=ts[:, k * 128:(k + 1) * 128],
                                    rhs=w2sb[:, k, :],
                                    start=(k == 0), stop=(k == KH - 1))
                            for k in range(KH):
                                nc.tensor.matmul(
                                    out=ps_h2[:, OUT:F2],
                                    lhsT=ts[:, (KH + k) * 128:(KH + k + 1) * 128],
                                    rhs=w2sb[:, k, :],
                                    start=(k == 0), stop=(k == KH - 1))
                            # layer-2 table rows: dinv^2 * (relu(s1) @ W2)
                            # (postponed layer-1 dinv_dst and layer-2 src factor)
                            hw2sb = sp.tile([128, F2], GDT2, tag="hw2sb",
                                            name=f"hw2sb_{b}")
                            nc.scalar.activation(
                                out=hw2sb[:], in_=ps_h2[:], func=ACT.Copy,
                                scale=dv2sb[:, b:b + 1] if bsb is None
                                else dvsb[:, b:b + 1])
                            nc.sync.dma_start(
                                out=hw2_sh[b * 128:(b + 1) * 128, :],
                                in_=hw2sb[:])
                        else:
                            hout = sp.tile([128, F2], HDT, tag="hout2",
                                           name=f"ho2_{b}")
                            nc.scalar.activation(
                                out=hout[:], in_=ps_agg[:, :F2], func=ACT.Copy,
                                scale=dvsb[:, b:b + 1])
                            if bsb is not None:
                                nc.vector.tensor_tensor(
                                    out=hout[:], in0=hout[:], in1=bsb[:],
                                    op=mybir.AluOpType.add)
                            if b == 0:
                                ps_cs = psC.tile([128, 1], F32, tag="cs",
                                                 space="PSUM")
                                agg_layer.cs = ps_cs
                            else:
                                ps_cs = agg_layer.cs
                            nc.tensor.matmul(
                                out=ps_cs[:], lhsT=hout[:, 0:OUT],
                                rhs=vdsb[:, b:b + 1],
                                start=(b == 0), stop=(b == NB - 1),
                                skip_group_check=True)
                            nc.sync.dma_start(
                                out=h2d[:, b * F2:(b + 1) * F2], in_=hout[:])

            agg_layer(1)
            allgather(hw2_sh, hw2_full, F2, GDT2)
            agg_layer(2)

            # ---------------- summary s and v = Wb @ s ----------------
            cssb = sp.tile([128, 1], F32, tag="cssb")
            nc.scalar.copy(out=cssb[:], in_=agg_layer.cs[:])
            nc.sync.dma_start(out=cs_in[:, :], in_=cssb[:])
            if LOCAL_SIM:
                nc.sync.dma_start(out=cs_out[:, :], in_=cssb[:])
            else:
                nc.gpsimd.collective_compute(
                    "AllReduce", mybir.AluOpType.add,
                    replica_groups=[list(range(C))],
                    ins=[cs_in[:, :].opt()], outs=[cs_out[:, :].opt()])
            csr = sp.tile([128, 1], F32, tag="csr")
            nc.sync.dma_start(out=csr[:], in_=cs_out[:, :])
            ssb = sp.tile([128, 1], F32, tag="ssb")
            nc.scalar.activation(out=ssb[:], in_=csr[:], func=ACT.Sigmoid,
                                 scale=1.0 / N)
            ps_v = psC.tile([128, 1], F32, tag="cs", space="PSUM")
            nc.tensor.matmul(out=ps_v[:], lhsT=wbtsb[:], rhs=ssb[:],
                             start=True, stop=True)
            vsb = sp.tile([128, 1], F32, tag="vsb")
            nc.scalar.copy(out=vsb[:], in_=ps_v[:])
            # v^T broadcast to all partitions: vbc[p, f] = v[f]
            ps_vt = psH.tile([128, F2], F32, tag="h2", space="PSUM")
            nc.tensor.transpose(out=ps_vt[0:1, 0:128], in_=vsb[:, 0:1],
                                identity=identf[:])
            vrow = sp.tile([128, 128], F32, tag="vrow")
            nc.scalar.copy(out=vrow[0:1, :], in_=ps_vt[0:1, 0:128])
            nc.gpsimd.partition_broadcast(out_ap=vbc[:], in_ap=vrow[0:1, :])

            # ---------------- scores: sc[p, b] = H[p, b, :] . v ----------
            SCB = 8
            for sb0 in range(0, NB, SCB):
                span = min(SCB, NB - sb0)
                hld = sp.tile([128, SCB * F2], HDT, tag="hld")
                nc.sync.dma_start(out=hld[:, :span * F2],
                                  in_=h2d[:, sb0 * F2:(sb0 + span) * F2])
                for j in range(span):
                    b = sb0 + j
                    prod = sp.tile([128, F2], F32, tag="prod")
                    nc.vector.tensor_tensor(
                        out=prod[:, 0:128], in0=hld[:, j * F2:j * F2 + 128],
                        in1=vbc[:], op=mybir.AluOpType.mult)
                    nc.vector.tensor_tensor(
                        out=prod[:, 128:F2],
                        in0=hld[:, j * F2 + 128:(j + 1) * F2],
                        in1=vbc[:], op=mybir.AluOpType.mult)
                    nc.vector.reduce_sum(out=sc_pos[:, b:b + 1],
                                         in_=prod[:, 0:128],
                                         axis=mybir.AxisListType.X)
                    nc.vector.reduce_sum(out=sc_neg[:, b:b + 1],
                                         in_=prod[:, 128:F2],
                                         axis=mybir.AxisListType.X)
            if bb_val != 0.0:
                nc.scalar.activation(out=sc_pos[:], in_=sc_pos[:],
                                     func=ACT.Copy, bias=float(bb_val))
                nc.scalar.activation(out=sc_neg[:], in_=sc_neg[:],
                                     func=ACT.Copy, bias=float(bb_val))
            nc.sync.dma_start(out=out[0, :, :], in_=sc_pos[:])
            nc.sync.dma_start(out=out[1, :, :], in_=sc_neg[:])

    nc.compile()
    return nc


# ----------------------------------------------------------------------------
# entry point
# ----------------------------------------------------------------------------

_CACHE = {}


def _get_program(meta, HID, OUT, bias1_nz, bias2_nz, bb_val, C):
    key = (meta["N"], meta["E"], meta["IN"], HID, OUT, bias1_nz, bias2_nz,
           float(bb_val), C, meta["TOT"], meta["K"].tobytes())
    if key not in _CACHE:
        _CACHE[key] = _build(meta, HID, OUT, bias1_nz, bias2_nz, bb_val, C)
    return _CACHE[key]


def _make_in_maps(meta, arrs, W1, b1, W2, b2, Wb, C, bias1_nz, bias2_nz):
    wdt = mybir.dt.np(BF16)
    W1, W2 = W1.astype(wdt), W2.astype(wdt)
    in_maps = []
    for c in range(C):
        m = {
            "xtp": arrs["xT_pos"][c], "xtn": arrs["xT_neg"][c],
            "w1": W1, "w2": W2, "wbt": np.ascontiguousarray(Wb.T),
            "idx16": arrs["idx_dev"][c], "monehot": arrs["m_dev"][c],
            "dinv": arrs["dv_dev"][c], "dinv2": arrs["dv2_dev"][c],
            "valid": arrs["vd_dev"][c],
        }
        if bias1_nz:
            m["b1bc"] = np.tile(np.concatenate([b1, b1])[None, :], (128, 1))
        if bias2_nz:
            m["b2bc"] = np.tile(np.concatenate([b2, b2])[None, :], (128, 1))
        in_maps.append(m)
    return in_maps


def _unpack(meta, results):
    N = meta["N"]
    core, blk, slot = meta["core"], meta["blk"], meta["slot"]
    pos = np.empty((N, 1), np.float32)
    neg = np.empty((N, 1), np.float32)
    sc0 = np.stack([np.asarray(results[c]["scores"][0]) for c in range(len(results))])
    sc1 = np.stack([np.asarray(results[c]["scores"][1]) for c in range(len(results))])
    pos[:, 0] = sc0[core, slot, blk]
    neg[:, 0] = sc1[core, slot, blk]
    return pos, neg


def kernel(x, edge_index, perm, W1, b1, W2, b2, Wb, bb):
    C = 8
    x = np.asarray(x, np.float32)
    W1 = np.asarray(W1, np.float32)
    W2 = np.asarray(W2, np.float32)
    Wb = np.asarray(Wb, np.float32)
    b1 = np.asarray(b1, np.float32)
    b2 = np.asarray(b2, np.float32)
    bb_val = float(np.asarray(bb).reshape(-1)[0])
    HID = W1.shape[1]
    OUT = W2.shape[1]

    meta, arrs = _prep(x, edge_index, perm, C)
    bias1_nz = bool(np.any(b1))
    bias2_nz = bool(np.any(b2))
    nc = _get_program(meta, HID, OUT, bias1_nz, bias2_nz, bb_val, C)
    in_maps = _make_in_maps(meta, arrs, W1, b1, W2, b2, Wb, C, bias1_nz, bias2_nz)

    res = bass_utils.run_bass_kernel_spmd(nc, in_maps, core_ids=list(range(C)))
    return _unpack(meta, res.results)
